# revision 9
# baseline (speedup 1.0000x reference)
"""MixtureLinear Trainium2 kernel.

Computes, for B=256, IN=1024, OUT=1024, RANK=16:
    out[b,o] = sum_i input[b,i] * sum_r weight[o,i,r] * coef[b,r]
             + sum_r bias[o,r] * coef[b,r]

Strategy (8 NeuronCores, tensor-parallel on OUT):
  - Core c owns OUT rows [128c, 128c+128). It reads only its weight shard
    (1/8 of the 64MB weight tensor), input/coef replicated.
  - Stage 1 (PE): proj[b,(o,r)] = inputT.T @ W2 where W2[i, o*16+r] =
    weight[o,i,r]; K=IN accumulated over 8 psum matmuls per 512-column
    chunk (one psum bank, 32 o's x 16 r's per chunk).
  - Stage 2 (DVE): out[b,o] = sum_r proj[b,(o,r)] * coef[b,r] via a
    broadcast-AP multiply + strided reduce over the innermost rank axis.
  - Bias: one tiny K=16 matmul per b-chunk: coefT.T @ biasT -> psum,
    added in the final DVE add before the output DMA.

Matmul dtype is selectable via MIXL_DT (float16 default; bfloat16 /
float32r / float32 supported). Host pre-casts and pre-transposes shards;
stage-2 and all accumulation stay fp32.
"""

import os
import sys
from contextlib import ExitStack

sys.path.insert(0, "/opt/trn_rl_repo")

import numpy as np
import ml_dtypes

import concourse.bass as bass
import concourse.tile as tile
from concourse import bacc, mybir
from concourse.bass_utils import run_bass_kernel_spmd

B, IN, OUT, RANK = 256, 1024, 1024, 16
NCORES = 8
OUTL = OUT // NCORES        # 128 out rows per core
P = 128                     # partitions
NB = B // P                 # 2 batch chunks
NK = IN // P                # 8 contraction chunks
CH = 512                    # psum chunk: one fp32 bank
NCH = OUTL * RANK // CH     # 4 column chunks per core
OCH = CH // RANK            # 32 o's per chunk

DT_NAME = os.environ.get("MIXL_DT", "float16")

_DT_MAP = {
    "float16": (mybir.dt.float16, np.float16),
    "bfloat16": (mybir.dt.bfloat16, ml_dtypes.bfloat16),
    "float32r": (mybir.dt.float32r, np.float32),
    "float32": (mybir.dt.float32, np.float32),
}


def build_nc(dt_name=DT_NAME):
    dt, _ = _DT_MAP[dt_name]
    f32 = mybir.dt.float32
    # Bacc (not raw Bass): its compile() runs generate_event_semaphores,
    # which splits multi-wait sync_info into EventSemaphore prefixes —
    # walrus accepts at most one wait per regular instruction.
    nc = bacc.Bacc("TRN2", target_bir_lowering=False, debug=False)

    xT = nc.declare_dram_parameter("xT", [IN, B], dt, isOutput=False)
    w2 = nc.declare_dram_parameter("w2", [NCH, IN, CH], dt, isOutput=False)
    coef = nc.declare_dram_parameter("coef", [B, RANK], f32, isOutput=False)
    coefT = nc.declare_dram_parameter("coefT", [RANK, B], dt, isOutput=False)
    biasT = nc.declare_dram_parameter("biasT", [RANK, OUTL], dt, isOutput=False)
    out = nc.declare_dram_parameter("out", [B, OUTL], f32, isOutput=True)

    with tile.TileContext(nc) as tc, ExitStack() as ctx:
        cpool = ctx.enter_context(tc.tile_pool(name="const", bufs=1))
        # bufs = total weight tiles: every tile gets a unique slot so the
        # HWDGE weight DMAs never need a slot-reuse wait (this walrus
        # supports only ONE sync-wait per HWDGE DMA trigger).
        wpool = ctx.enter_context(tc.tile_pool(name="w", bufs=NCH * NK))
        ppool = ctx.enter_context(tc.tile_pool(name="proj", bufs=4, space="PSUM"))
        bpool = ctx.enter_context(tc.tile_pool(name="biasps", bufs=2, space="PSUM"))
        spool = ctx.enter_context(tc.tile_pool(name="stage2", bufs=4))
        opool = ctx.enter_context(tc.tile_pool(name="outp", bufs=2))

        # Constants: full inputT (all K chunks), coef (per-partition scalars),
        # coefT + biasT for the bias matmul.
        xT_t = cpool.tile([P, NK, B], dt, tag="xT")
        nc.sync.dma_start(xT_t[:], xT.rearrange("(k p) b -> p k b", p=P))
        coef_t = cpool.tile([P, NB, RANK], f32, tag="coef")
        nc.sync.dma_start(coef_t[:], coef.rearrange("(nb p) r -> p nb r", p=P))
        coefT_t = cpool.tile([RANK, B], dt, tag="coefT")
        nc.sync.dma_start(coefT_t[:], coefT[:])
        biasT_t = cpool.tile([RANK, OUTL], dt, tag="biasT")
        nc.sync.dma_start(biasT_t[:], biasT[:])

        # Bias term: out_bias[b,o] = sum_r coef[b,r] * bias[o,r]
        bias_ps = []
        for b in range(NB):
            bp = bpool.tile([P, OUTL], f32, tag="bias")
            nc.tensor.matmul(
                bp[:], lhsT=coefT_t[:, b * P:(b + 1) * P], rhs=biasT_t[:],
                start=True, stop=True,
            )
            bias_ps.append(bp)

        out_sb = [
            opool.tile([P, OUTL], f32, tag="osum", name=f"osum{b}")
            for b in range(NB)
        ]

        for n in range(NCH):
            pss = [
                ppool.tile([P, CH], f32, tag="proj", name=f"proj{n}_{b}")
                for b in range(NB)
            ]
            for k in range(NK):
                wt = wpool.tile([P, CH], dt, tag="w")
                nc.sync.dma_start(wt[:], w2[n, k * P:(k + 1) * P, :])
                for b in range(NB):
                    nc.tensor.matmul(
                        pss[b][:],
                        lhsT=xT_t[:, k, b * P:(b + 1) * P],
                        rhs=wt[:],
                        start=(k == 0),
                        stop=(k == NK - 1),
                    )
            # Rank contraction: multiply by per-(b,r) coef, reduce over r.
            for b in range(NB):
                tmp = spool.tile([P, CH], f32, tag="tmp")
                coef_b = coef_t[:, b, :].rearrange("p (one r) -> p one r", one=1)
                nc.vector.tensor_mul(
                    tmp[:].rearrange("p (o r) -> p o r", r=RANK),
                    pss[b][:].rearrange("p (o r) -> p o r", r=RANK),
                    coef_b.to_broadcast((P, OCH, RANK)),
                )
                nc.vector.tensor_reduce(
                    out_sb[b][:, n * OCH:(n + 1) * OCH],
                    tmp[:].rearrange("p (o r) -> p o r", r=RANK),
                    axis=mybir.AxisListType.X,
                    op=mybir.AluOpType.add,
                )

        for b in range(NB):
            outf = opool.tile([P, OUTL], f32, tag="outf")
            nc.vector.tensor_add(outf[:], out_sb[b][:], bias_ps[b][:])
            nc.sync.dma_start(out[b * P:(b + 1) * P, :], outf[:])

    nc.compile()
    return nc


def prepare_in_maps(input, coef, weight, bias, dt_name=DT_NAME):
    _, npdt = _DT_MAP[dt_name]
    xT = np.ascontiguousarray(input.T).astype(npdt)          # (IN, B)
    coefT = np.ascontiguousarray(coef.T).astype(npdt)        # (RANK, B)
    coef32 = np.ascontiguousarray(coef.astype(np.float32))   # (B, RANK)
    in_maps = []
    for c in range(NCORES):
        wsh = weight[c * OUTL:(c + 1) * OUTL]                # (OUTL, IN, RANK)
        # W2[i, o*RANK+r] = wsh[o, i, r], then n-major chunks of 512 cols
        w2 = wsh.transpose(1, 0, 2).reshape(IN, OUTL * RANK)
        w2 = np.ascontiguousarray(
            w2.reshape(IN, NCH, CH).transpose(1, 0, 2)
        ).astype(npdt)                                       # (NCH, IN, CH)
        biasT = np.ascontiguousarray(
            bias[c * OUTL:(c + 1) * OUTL].T
        ).astype(npdt)                                       # (RANK, OUTL)
        in_maps.append({
            "xT": xT, "w2": w2, "coef": coef32,
            "coefT": coefT, "biasT": biasT,
        })
    return in_maps


_NC_CACHE = {}


def _ensure_ntff_hook():
    """The agent image's antenv lacks axon_hooks; inject it and register
    the ctypes NTFF profile hook so trace=True works under axon."""
    import types
    import antenv
    try:
        from antenv import axon_hooks  # noqa: F401
        return
    except ImportError:
        pass
    mod = types.ModuleType("antenv.axon_hooks")
    _state = {"hook": None}
    mod.set_axon_ntff_profile_hook = lambda h: _state.__setitem__("hook", h)
    mod.get_axon_ntff_profile_hook = lambda: _state["hook"]
    sys.modules["antenv.axon_hooks"] = mod
    antenv.axon_hooks = mod
    try:
        from trn_agent_boot.trn_boot import _ntff_profile_via_ctypes
        mod.set_axon_ntff_profile_hook(
            _ntff_profile_via_ctypes("/opt/axon/libaxon_pjrt.so")
        )
    except Exception:
        pass


def run(inputs, trace=False, dt_name=DT_NAME, **kwargs):
    if trace:
        _ensure_ntff_hook()
    if dt_name not in _NC_CACHE:
        _NC_CACHE[dt_name] = build_nc(dt_name)
    nc = _NC_CACHE[dt_name]
    in_maps = prepare_in_maps(
        np.asarray(inputs["input"], dtype=np.float32),
        np.asarray(inputs["coef"], dtype=np.float32),
        np.asarray(inputs["weight"], dtype=np.float32),
        np.asarray(inputs["bias"], dtype=np.float32),
        dt_name,
    )
    br = run_bass_kernel_spmd(
        nc, in_maps, list(range(NCORES)), trace=trace, **kwargs
    )
    full = np.concatenate(
        [br.results[c]["out"] for c in range(NCORES)], axis=1
    ).astype(np.float32)
    return full, br


def kernel(**inputs):
    full, _ = run(inputs)
    return full


# revision 11
# speedup vs baseline: 1.1918x; 1.1918x over previous
"""MixtureLinear Trainium2 kernel.

Computes, for B=256, IN=1024, OUT=1024, RANK=16:
    out[b,o] = sum_i input[b,i] * sum_r weight[o,i,r] * coef[b,r]
             + sum_r bias[o,r] * coef[b,r]

Strategy (8 NeuronCores, tensor-parallel on OUT):
  - Core c owns OUT rows [128c, 128c+128). It reads only its weight shard
    (1/8 of the 64MB weight tensor), input/coef replicated.
  - Stage 1 (PE): proj[b,(o,r)] = inputT.T @ W2 where W2[i, o*16+r] =
    weight[o,i,r]; K=IN accumulated over 8 psum matmuls per 512-column
    chunk (one psum bank, 32 o's x 16 r's per chunk).
  - Stage 2 (DVE): out[b,o] = sum_r proj[b,(o,r)] * coef[b,r] via a
    broadcast-AP multiply + strided reduce over the innermost rank axis.
  - Bias: one tiny K=16 matmul per b-chunk: coefT.T @ biasT -> psum,
    added in the final DVE add before the output DMA.

Matmul dtype is selectable via MIXL_DT (float16 default; bfloat16 /
float32r / float32 supported). Host pre-casts and pre-transposes shards;
stage-2 and all accumulation stay fp32.
"""

import os
import sys
from contextlib import ExitStack

sys.path.insert(0, "/opt/trn_rl_repo")

import numpy as np
import ml_dtypes

import concourse.bass as bass
import concourse.tile as tile
from concourse import bacc, mybir
from concourse.bass_utils import run_bass_kernel_spmd

B, IN, OUT, RANK = 256, 1024, 1024, 16
NCORES = 8
OUTL = OUT // NCORES        # 128 out rows per core
P = 128                     # partitions
NB = B // P                 # 2 batch chunks
NK = IN // P                # 8 contraction chunks
CH = 512                    # psum chunk: one fp32 bank
NCH = OUTL * RANK // CH     # 4 column chunks per core
OCH = CH // RANK            # 32 o's per chunk

DT_NAME = os.environ.get("MIXL_DT", "float16")

_DT_MAP = {
    "float16": (mybir.dt.float16, np.float16),
    "bfloat16": (mybir.dt.bfloat16, ml_dtypes.bfloat16),
    "float32r": (mybir.dt.float32r, np.float32),
    "float32": (mybir.dt.float32, np.float32),
}


def build_nc(dt_name=DT_NAME):
    dt, _ = _DT_MAP[dt_name]
    f32 = mybir.dt.float32
    # Bacc (not raw Bass): its compile() runs generate_event_semaphores,
    # which splits multi-wait sync_info into EventSemaphore prefixes —
    # walrus accepts at most one wait per regular instruction.
    nc = bacc.Bacc("TRN2", target_bir_lowering=False, debug=False)

    xT = nc.declare_dram_parameter("xT", [IN, B], dt, isOutput=False)
    w2 = nc.declare_dram_parameter("w2", [NCH, IN, CH], dt, isOutput=False)
    coef = nc.declare_dram_parameter("coef", [B, RANK], f32, isOutput=False)
    coefT = nc.declare_dram_parameter("coefT", [RANK, B], dt, isOutput=False)
    biasT = nc.declare_dram_parameter("biasT", [RANK, OUTL], dt, isOutput=False)
    out = nc.declare_dram_parameter("out", [B, OUTL], f32, isOutput=True)

    with tile.TileContext(nc) as tc, ExitStack() as ctx:
        cpool = ctx.enter_context(tc.tile_pool(name="const", bufs=1))
        wpool = ctx.enter_context(tc.tile_pool(name="w", bufs=3))
        ppool = ctx.enter_context(tc.tile_pool(name="proj", bufs=4, space="PSUM"))
        bpool = ctx.enter_context(tc.tile_pool(name="biasps", bufs=2, space="PSUM"))
        spool = ctx.enter_context(tc.tile_pool(name="stage2", bufs=4))
        opool = ctx.enter_context(tc.tile_pool(name="outp", bufs=2))

        # Constants: full inputT (all K chunks), coef (per-partition scalars),
        # coefT + biasT for the bias matmul.
        xT_t = cpool.tile([P, NK, B], dt, tag="xT")
        nc.sync.dma_start(xT_t[:], xT.rearrange("(k p) b -> p k b", p=P))
        coef_t = cpool.tile([P, NB, RANK], f32, tag="coef")
        nc.sync.dma_start(coef_t[:], coef.rearrange("(nb p) r -> p nb r", p=P))
        coefT_t = cpool.tile([RANK, B], dt, tag="coefT")
        nc.sync.dma_start(coefT_t[:], coefT[:])
        biasT_t = cpool.tile([RANK, OUTL], dt, tag="biasT")
        nc.sync.dma_start(biasT_t[:], biasT[:])

        # Bias term: out_bias[b,o] = sum_r coef[b,r] * bias[o,r]
        bias_ps = []
        for b in range(NB):
            bp = bpool.tile([P, OUTL], f32, tag="bias")
            nc.tensor.matmul(
                bp[:], lhsT=coefT_t[:, b * P:(b + 1) * P], rhs=biasT_t[:],
                start=True, stop=True,
            )
            bias_ps.append(bp)

        out_sb = [
            opool.tile([P, OUTL], f32, tag="osum", name=f"osum{b}")
            for b in range(NB)
        ]

        for n in range(NCH):
            pss = [
                ppool.tile([P, CH], f32, tag="proj", name=f"proj{n}_{b}")
                for b in range(NB)
            ]
            # One 1MB DMA per n-chunk: small (128K) transfers only reach
            # ~50% of HBM bandwidth and starve the PE (measured 177 GB/s,
            # PE HAM-throttled most of the run).
            wt = wpool.tile([P, NK, CH], dt, tag="w")
            nc.sync.dma_start(wt[:], w2[n].rearrange("(k p) c -> p k c", p=P))
            for k in range(NK):
                for b in range(NB):
                    nc.tensor.matmul(
                        pss[b][:],
                        lhsT=xT_t[:, k, b * P:(b + 1) * P],
                        rhs=wt[:, k, :],
                        start=(k == 0),
                        stop=(k == NK - 1),
                    )
            # Rank contraction: multiply by per-(b,r) coef, reduce over r.
            for b in range(NB):
                tmp = spool.tile([P, CH], f32, tag="tmp")
                coef_b = coef_t[:, b, :].rearrange("p (one r) -> p one r", one=1)
                nc.vector.tensor_mul(
                    tmp[:].rearrange("p (o r) -> p o r", r=RANK),
                    pss[b][:].rearrange("p (o r) -> p o r", r=RANK),
                    coef_b.to_broadcast((P, OCH, RANK)),
                )
                nc.vector.tensor_reduce(
                    out_sb[b][:, n * OCH:(n + 1) * OCH],
                    tmp[:].rearrange("p (o r) -> p o r", r=RANK),
                    axis=mybir.AxisListType.X,
                    op=mybir.AluOpType.add,
                )

        for b in range(NB):
            outf = opool.tile([P, OUTL], f32, tag="outf")
            nc.vector.tensor_add(outf[:], out_sb[b][:], bias_ps[b][:])
            nc.sync.dma_start(out[b * P:(b + 1) * P, :], outf[:])

    nc.compile()
    return nc


def prepare_in_maps(input, coef, weight, bias, dt_name=DT_NAME):
    _, npdt = _DT_MAP[dt_name]
    xT = np.ascontiguousarray(input.T).astype(npdt)          # (IN, B)
    coefT = np.ascontiguousarray(coef.T).astype(npdt)        # (RANK, B)
    coef32 = np.ascontiguousarray(coef.astype(np.float32))   # (B, RANK)
    in_maps = []
    for c in range(NCORES):
        wsh = weight[c * OUTL:(c + 1) * OUTL]                # (OUTL, IN, RANK)
        # W2[i, o*RANK+r] = wsh[o, i, r], then n-major chunks of 512 cols
        w2 = wsh.transpose(1, 0, 2).reshape(IN, OUTL * RANK)
        w2 = np.ascontiguousarray(
            w2.reshape(IN, NCH, CH).transpose(1, 0, 2)
        ).astype(npdt)                                       # (NCH, IN, CH)
        biasT = np.ascontiguousarray(
            bias[c * OUTL:(c + 1) * OUTL].T
        ).astype(npdt)                                       # (RANK, OUTL)
        in_maps.append({
            "xT": xT, "w2": w2, "coef": coef32,
            "coefT": coefT, "biasT": biasT,
        })
    return in_maps


_NC_CACHE = {}


def _ensure_ntff_hook():
    """The agent image's antenv lacks axon_hooks; inject it and register
    the ctypes NTFF profile hook so trace=True works under axon."""
    import types
    import antenv
    try:
        from antenv import axon_hooks  # noqa: F401
        return
    except ImportError:
        pass
    mod = types.ModuleType("antenv.axon_hooks")
    _state = {"hook": None}
    mod.set_axon_ntff_profile_hook = lambda h: _state.__setitem__("hook", h)
    mod.get_axon_ntff_profile_hook = lambda: _state["hook"]
    sys.modules["antenv.axon_hooks"] = mod
    antenv.axon_hooks = mod
    try:
        from trn_agent_boot.trn_boot import _ntff_profile_via_ctypes
        mod.set_axon_ntff_profile_hook(
            _ntff_profile_via_ctypes("/opt/axon/libaxon_pjrt.so")
        )
    except Exception:
        pass


def run(inputs, trace=False, dt_name=DT_NAME, **kwargs):
    if trace:
        _ensure_ntff_hook()
    if dt_name not in _NC_CACHE:
        _NC_CACHE[dt_name] = build_nc(dt_name)
    nc = _NC_CACHE[dt_name]
    in_maps = prepare_in_maps(
        np.asarray(inputs["input"], dtype=np.float32),
        np.asarray(inputs["coef"], dtype=np.float32),
        np.asarray(inputs["weight"], dtype=np.float32),
        np.asarray(inputs["bias"], dtype=np.float32),
        dt_name,
    )
    br = run_bass_kernel_spmd(
        nc, in_maps, list(range(NCORES)), trace=trace, **kwargs
    )
    full = np.concatenate(
        [br.results[c]["out"] for c in range(NCORES)], axis=1
    ).astype(np.float32)
    return full, br


def kernel(**inputs):
    full, _ = run(inputs)
    return full


# revision 16
# speedup vs baseline: 1.2224x; 1.0257x over previous
"""MixtureLinear Trainium2 kernel.

Computes, for B=256, IN=1024, OUT=1024, RANK=16:
    out[b,o] = sum_i input[b,i] * sum_r weight[o,i,r] * coef[b,r]
             + sum_r bias[o,r] * coef[b,r]

Strategy (8 NeuronCores, tensor-parallel on OUT):
  - Core c owns OUT rows [128c, 128c+128). It reads only its weight shard
    (1/8 of the 64MB weight tensor), input/coef replicated.
  - Stage 1 (PE): proj[b,(o,r)] = inputT.T @ W2 where W2[i, o*16+r] =
    weight[o,i,r]; K=IN accumulated over 8 psum matmuls per 512-column
    chunk (one psum bank, 32 o's x 16 r's per chunk).
  - Stage 2 (DVE): out[b,o] = sum_r proj[b,(o,r)] * coef[b,r] via a
    broadcast-AP multiply + strided reduce over the innermost rank axis.
  - Bias: one tiny K=16 matmul per b-chunk: coefT.T @ biasT -> psum,
    added in the final DVE add before the output DMA.

Matmul dtype is selectable via MIXL_DT (float16 default; bfloat16 /
float32r / float32 supported). Host pre-casts and pre-transposes shards;
stage-2 and all accumulation stay fp32.
"""

import os
import sys
from contextlib import ExitStack

sys.path.insert(0, "/opt/trn_rl_repo")

import numpy as np
import ml_dtypes

import concourse.bass as bass
import concourse.tile as tile
from concourse import bacc, mybir
from concourse.bass_utils import run_bass_kernel_spmd

B, IN, OUT, RANK = 256, 1024, 1024, 16
NCORES = 8
OUTL = OUT // NCORES        # 128 out rows per core
P = 128                     # partitions
NB = B // P                 # 2 batch chunks
NK = IN // P                # 8 contraction chunks
CH = 512                    # psum chunk: one fp32 bank
NCH = OUTL * RANK // CH     # 4 column chunks per core
OCH = CH // RANK            # 32 o's per chunk

DT_NAME = os.environ.get("MIXL_DT", "float16")

_DT_MAP = {
    "float16": (mybir.dt.float16, np.float16),
    "bfloat16": (mybir.dt.bfloat16, ml_dtypes.bfloat16),
    "float32r": (mybir.dt.float32r, np.float32),
    "float32": (mybir.dt.float32, np.float32),
}


def build_nc(dt_name=DT_NAME):
    dt, _ = _DT_MAP[dt_name]
    f32 = mybir.dt.float32
    # Bacc (not raw Bass): its compile() runs generate_event_semaphores,
    # which splits multi-wait sync_info into EventSemaphore prefixes —
    # walrus accepts at most one wait per regular instruction.
    nc = bacc.Bacc("TRN2", target_bir_lowering=False, debug=False)

    xT = nc.declare_dram_parameter("xT", [IN, B], dt, isOutput=False)
    # w2[n, p, k*CH+c] = W2[k*128+p, n*CH+c]: pre-swizzled on host so each
    # SBUF partition's data is one contiguous 8KB run in DRAM (full-rate DMA).
    w2 = nc.declare_dram_parameter("w2", [NCH, P, NK * CH], dt, isOutput=False)
    coef = nc.declare_dram_parameter("coef", [B, RANK], f32, isOutput=False)
    coefT = nc.declare_dram_parameter("coefT", [RANK, B], dt, isOutput=False)
    biasT = nc.declare_dram_parameter("biasT", [RANK, OUTL], dt, isOutput=False)
    out = nc.declare_dram_parameter("out", [B, OUTL], f32, isOutput=True)

    with tile.TileContext(nc) as tc, ExitStack() as ctx:
        cpool = ctx.enter_context(tc.tile_pool(name="const", bufs=1))
        wpool = ctx.enter_context(tc.tile_pool(name="w", bufs=NCH))
        ppool = ctx.enter_context(tc.tile_pool(name="proj", bufs=6, space="PSUM"))
        bpool = ctx.enter_context(tc.tile_pool(name="biasps", bufs=2, space="PSUM"))
        spool = ctx.enter_context(tc.tile_pool(name="stage2", bufs=4))
        opool = ctx.enter_context(tc.tile_pool(name="outp", bufs=2))

        # Weight tiles for every n-chunk (issued first; n=0 split so the
        # first matmuls can start after only 256KB has landed).
        wts = [wpool.tile([P, NK, CH], dt, tag="w", name=f"wt{n}")
               for n in range(NCH)]
        w2v = w2.rearrange("n p (k c) -> n p k c", c=CH)
        nc.sync.dma_start(wts[0][:, 0:2, :], w2v[0][:, 0:2, :])
        # Full inputT, split in halves (first matmuls need only low k).
        xT_t = cpool.tile([P, NK, B], dt, tag="xT")
        xTv = xT.rearrange("(k p) b -> p k b", p=P)
        nc.sync.dma_start(xT_t[:, 0:NK // 2, :], xTv[:, 0:NK // 2, :])
        nc.sync.dma_start(wts[0][:, 2:NK, :], w2v[0][:, 2:NK, :])
        nc.sync.dma_start(xT_t[:, NK // 2:, :], xTv[:, NK // 2:, :])
        for n in range(1, NCH):
            nc.sync.dma_start(wts[n][:], w2[n].rearrange("p (k c) -> p k c", c=CH))
        coef_t = cpool.tile([P, NB, RANK], f32, tag="coef")
        nc.sync.dma_start(coef_t[:], coef.rearrange("(nb p) r -> p nb r", p=P))
        coefT_t = cpool.tile([RANK, B], dt, tag="coefT")
        nc.sync.dma_start(coefT_t[:], coefT[:])
        biasT_t = cpool.tile([RANK, OUTL], dt, tag="biasT")
        nc.sync.dma_start(biasT_t[:], biasT[:])

        # Bias term: out_bias[b,o] = sum_r coef[b,r] * bias[o,r]
        bias_ps = []
        for b in range(NB):
            bp = bpool.tile([P, OUTL], f32, tag="bias")
            nc.tensor.matmul(
                bp[:], lhsT=coefT_t[:, b * P:(b + 1) * P], rhs=biasT_t[:],
                start=True, stop=True,
            )
            bias_ps.append(bp)

        out_sb = [
            opool.tile([P, OUTL], f32, tag="osum", name=f"osum{b}")
            for b in range(NB)
        ]

        for n in range(NCH):
            pss = [
                ppool.tile([P, CH], f32, tag="proj", name=f"proj{n}_{b}")
                for b in range(NB)
            ]
            wt = wts[n]
            for k in range(NK):
                for b in range(NB):
                    nc.tensor.matmul(
                        pss[b][:],
                        lhsT=xT_t[:, k, b * P:(b + 1) * P],
                        rhs=wt[:, k, :],
                        start=(k == 0),
                        stop=(k == NK - 1),
                    )
            # Rank contraction: multiply by per-(b,r) coef, reduce over r.
            for b in range(NB):
                tmp = spool.tile([P, CH], f32, tag="tmp")
                coef_b = coef_t[:, b, :].rearrange("p (one r) -> p one r", one=1)
                nc.vector.tensor_mul(
                    tmp[:].rearrange("p (o r) -> p o r", r=RANK),
                    pss[b][:].rearrange("p (o r) -> p o r", r=RANK),
                    coef_b.to_broadcast((P, OCH, RANK)),
                )
                nc.vector.tensor_reduce(
                    out_sb[b][:, n * OCH:(n + 1) * OCH],
                    tmp[:].rearrange("p (o r) -> p o r", r=RANK),
                    axis=mybir.AxisListType.X,
                    op=mybir.AluOpType.add,
                )

        for b in range(NB):
            outf = opool.tile([P, OUTL], f32, tag="outf")
            nc.vector.tensor_add(outf[:], out_sb[b][:], bias_ps[b][:])
            nc.sync.dma_start(out[b * P:(b + 1) * P, :], outf[:])

    nc.compile()
    return nc


def prepare_in_maps(input, coef, weight, bias, dt_name=DT_NAME):
    _, npdt = _DT_MAP[dt_name]
    xT = np.ascontiguousarray(input.T).astype(npdt)          # (IN, B)
    coefT = np.ascontiguousarray(coef.T).astype(npdt)        # (RANK, B)
    coef32 = np.ascontiguousarray(coef.astype(np.float32))   # (B, RANK)
    in_maps = []
    for c in range(NCORES):
        wsh = weight[c * OUTL:(c + 1) * OUTL]                # (OUTL, IN, RANK)
        # W2[i, o*RANK+r] = wsh[o, i, r]; n-major 512-col chunks; then swizzle
        # (n, i=k*128+p, c) -> (n, p, k, c) so each partition reads one
        # contiguous 8KB run per n-chunk DMA.
        w2 = wsh.transpose(1, 0, 2).reshape(IN, OUTL * RANK)
        w2 = w2.reshape(NK, P, NCH, CH).transpose(2, 1, 0, 3)
        w2 = np.ascontiguousarray(w2.reshape(NCH, P, NK * CH)).astype(npdt)
        biasT = np.ascontiguousarray(
            bias[c * OUTL:(c + 1) * OUTL].T
        ).astype(npdt)                                       # (RANK, OUTL)
        in_maps.append({
            "xT": xT, "w2": w2, "coef": coef32,
            "coefT": coefT, "biasT": biasT,
        })
    return in_maps


_NC_CACHE = {}


def _ensure_ntff_hook():
    """The agent image's antenv lacks axon_hooks; inject it and register
    the ctypes NTFF profile hook so trace=True works under axon."""
    import types
    import antenv
    try:
        from antenv import axon_hooks  # noqa: F401
        return
    except ImportError:
        pass
    mod = types.ModuleType("antenv.axon_hooks")
    _state = {"hook": None}
    mod.set_axon_ntff_profile_hook = lambda h: _state.__setitem__("hook", h)
    mod.get_axon_ntff_profile_hook = lambda: _state["hook"]
    sys.modules["antenv.axon_hooks"] = mod
    antenv.axon_hooks = mod
    try:
        from trn_agent_boot.trn_boot import _ntff_profile_via_ctypes
        mod.set_axon_ntff_profile_hook(
            _ntff_profile_via_ctypes("/opt/axon/libaxon_pjrt.so")
        )
    except Exception:
        pass


def run(inputs, trace=False, dt_name=DT_NAME, **kwargs):
    if trace:
        _ensure_ntff_hook()
    if dt_name not in _NC_CACHE:
        _NC_CACHE[dt_name] = build_nc(dt_name)
    nc = _NC_CACHE[dt_name]
    in_maps = prepare_in_maps(
        np.asarray(inputs["input"], dtype=np.float32),
        np.asarray(inputs["coef"], dtype=np.float32),
        np.asarray(inputs["weight"], dtype=np.float32),
        np.asarray(inputs["bias"], dtype=np.float32),
        dt_name,
    )
    br = run_bass_kernel_spmd(
        nc, in_maps, list(range(NCORES)), trace=trace, **kwargs
    )
    full = np.concatenate(
        [br.results[c]["out"] for c in range(NCORES)], axis=1
    ).astype(np.float32)
    return full, br


def kernel(**inputs):
    full, _ = run(inputs)
    return full
